# revision 2
# baseline (speedup 1.0000x reference)
"""AttnReadout kernel for Trainium2 (8 NeuronCores, data-parallel over batch).

Computes, for x:[B,N,D], last_nodes:[B], W_u/W_v:[D,D], b_u:[D], W_e:[D,1]:
    x_u   = x @ W_u + b_u
    x_v   = x[b, last_nodes[b]] @ W_v
    e     = sigmoid(x_u + x_v[:,None]) @ W_e
    alpha = softmax(e, axis=-2)
    out   = sum(x * alpha, axis=-2)          # [B, D]

Sharding: batch is split 8 ways (32 batches per core); the tiny weights are
replicated. No cross-core communication.

x ships in two layouts: natural bf16 xn[b,p,h,c,d] = x[b, 4p+2h+c, d] (a pure
reshape; used for the alpha-weighted pooling and the x_last gather) and
transposed fp8 xt[b,p,c,n'] with n' = i*128+q <-> n = 4q+i (for the W_u
DoubleRow matmul).  The n-permutation is self-consistent between the two, so
softmax/pooling work entirely in n' order.

Engine/queue placement (HW-bisected; ScalarE is the bottleneck engine at
~43us/sweep of sigmoid work, and DMA trigger queues are delicate):
  - natural-x loads (4 batches, 1MB) go through nc.gpsimd (SWDGE) - the Pool
    engine is otherwise idle; putting them on nc.scalar stalls ScalarE
    (80.7us) and on nc.sync overloads the SP queue (65.5us).
  - transposed-x loads + small gathers/stores stay on nc.sync (HWDGE).
  - sigmoid outputs s and W_e are fp8 (the e-matvec contracts them exactly
    as bf16-rate fp8); probabilities/pooling stay bf16 for accuracy.
  - per-d-chunk sigmoid carries the per-partition x_v bias; W_u/W_v/W_e ship
    pre-scaled x16 so fp8 stays in normal range (undone by activation scales).
"""

import numpy as np
import ml_dtypes
from contextlib import ExitStack

try:
    import concourse.bass as bass
except ImportError:  # stock container: repo lives in /opt
    import sys

    sys.path.insert(0, "/opt/trn_rl_repo")
    import concourse.bass as bass

from concourse import bacc, mybir
import concourse.tile as tile
from concourse.bass_utils import run_bass_kernel_spmd

DT = mybir.dt
BF16 = DT.bfloat16
F32 = DT.float32
FP8 = DT.float8e4
WU_SCALE = 16.0  # W_u, W_v, b_u, W_e all pre-scaled x16 into fp8/bf16 range
B, N, D = 256, 512, 256
NCORES = 8
BSH = B // NCORES  # 32 batches per core
P = 128
NCH_N = N // P  # 4 chunks of the node axis
NCH_D = D // P  # 2 chunks of the feature axis
GRP = 32  # softmax batching group

Sigmoid = mybir.ActivationFunctionType.Sigmoid
Exp = mybir.ActivationFunctionType.Exp
Square = mybir.ActivationFunctionType.Square
DR = mybir.MatmulPerfMode.DoubleRow
Mult = mybir.AluOpType.mult
Add = mybir.AluOpType.add
AxX = mybir.AxisListType.X


def build_nc(nb=BSH, reps=1, grp=None, **_unused):
    GRP = grp or globals()["GRP"]
    nc = bacc.Bacc("TRN2", target_bir_lowering=False, debug=False, num_devices=NCORES)

    x_d = nc.dram_tensor("x", [nb, P, 2, 2, D], BF16, kind="ExternalInput")
    xt_d = nc.dram_tensor("xt", [nb, P, NCH_D, N], FP8, kind="ExternalInput")
    offs_d = nc.dram_tensor("offs", [nb, 1], DT.int32, kind="ExternalInput")
    wu_d = nc.dram_tensor("wu", [P, NCH_D, D], FP8, kind="ExternalInput")
    wv_d = nc.dram_tensor("wv", [P, NCH_D, D], BF16, kind="ExternalInput")
    we_d = nc.dram_tensor("we", [P, NCH_D, 32], FP8, kind="ExternalInput")
    bu_d = nc.dram_tensor("bu", [P, NCH_D], F32, kind="ExternalInput")
    idb_d = nc.dram_tensor("idb", [P, P], BF16, kind="ExternalInput")
    out_d = nc.dram_tensor("out", [nb, D], F32, kind="ExternalOutput")

    with tile.TileContext(nc) as tc, ExitStack() as ctx:
        consts = ctx.enter_context(tc.tile_pool(name="consts", bufs=1))
        # xb tiles (4 batches each) stay live across the one-group software
        # pipeline: 2 groups in flight plus loading margin
        nxb = nb // 4 + 3
        xnat = ctx.enter_context(tc.tile_pool(name="xnat", bufs=nxb))
        xtp = ctx.enter_context(tc.tile_pool(name="xt", bufs=4))
        spool = ctx.enter_context(tc.tile_pool(name="s", bufs=3))
        smallp = ctx.enter_context(tc.tile_pool(name="small", bufs=2))
        # PSUM budget: 4 (xu, one bank each) + 2 (e) + 2 (shared o/psm) = 8
        pxu = ctx.enter_context(tc.tile_pool(name="pxu", bufs=4, space="PSUM"))
        pe_p = ctx.enter_context(tc.tile_pool(name="pe", bufs=2, space="PSUM"))
        pmix = ctx.enter_context(tc.tile_pool(name="pmix", bufs=2, space="PSUM"))

        # ---- constants ----
        wu_sb = consts.tile([P, NCH_D, D], FP8, tag="wu")
        nc.sync.dma_start(wu_sb[:], wu_d.ap())
        wv_sb = consts.tile([P, NCH_D, D], BF16, tag="wv")
        nc.sync.dma_start(wv_sb[:], wv_d.ap())
        we_sb = consts.tile([P, NCH_D, 32], FP8, tag="we")
        nc.sync.dma_start(we_sb[:], we_d.ap())
        bu_sb = consts.tile([P, NCH_D], F32, tag="bu")
        nc.sync.dma_start(bu_sb[:], bu_d.ap())
        idb_sb = consts.tile([P, P], BF16, tag="idb")
        nc.sync.dma_start(idb_sb[:], idb_d.ap())
        offs_sb = consts.tile([nb, 1], DT.int32, tag="offs")
        nc.sync.dma_start(offs_sb[:], offs_d.ap())

        # ---- phase 0: gather x_last; xv_row = 16*(W_v^T x_last + b_u) as a
        # per-batch fp8 row at partition 0, for the rank-1 PSUM bias fold ----
        xlast = consts.tile([nb, D], BF16, tag="xlast")
        nc.gpsimd.indirect_dma_start(
            out=xlast[:],
            out_offset=None,
            in_=x_d.ap().rearrange("b p h c d -> (b p h c) d"),
            in_offset=bass.IndirectOffsetOnAxis(ap=offs_sb[:, :1], axis=0),
        )
        # transpose to [D, nb] (d-major) so the W_v matmul can contract over d
        xlt_ps = pmix.tile([P, NCH_D, nb], F32, tag="o")
        for c in range(NCH_D):
            nc.tensor.matmul(
                xlt_ps[:, c, :], xlast[:, c * P : (c + 1) * P], idb_sb[:nb, :nb]
            )
        xlt = consts.tile([P, NCH_D, nb], BF16, tag="xlt")
        nc.vector.tensor_copy(xlt[:], xlt_ps[:])

        xvb = consts.tile([P, NCH_D, nb], F32, tag="xvb")
        for j in range(NCH_D):
            xv_ps = pmix.tile([P, nb], F32, tag="o")
            for c in range(NCH_D):
                nc.tensor.matmul(
                    xv_ps[:],
                    wv_sb[:, c, j * P : (j + 1) * P],
                    xlt[:, c, :],
                    start=(c == 0),
                    stop=(c == NCH_D - 1),
                )
            # psum holds 16*xv (wv/bu shipped x16); the activation bias is
            # applied after its 1/16 input scale, so store unscaled xv here
            nc.vector.tensor_scalar(
                xvb[:, j, :], xv_ps[:], bu_sb[:, j : j + 1], 1.0 / WU_SCALE, Add, Mult
            )


        # ---- main loop ----
        # The "final" phase (softmax + weighted pooling) of each group is
        # emitted one group late: its softmax/p-transpose prologue goes in
        # front of the next group's batch loop, and its per-batch pooling
        # matvecs run between the next group's e-matvecs on DISJOINT PE
        # column groups, so the hardware streams them concurrently.

        def final_prologue(g, gb, e_all):
            # softmax over n, batched across the group.  e_all holds 16*e.
            # p = exp(e - m) ~ (1 + (e-m)/8)^8 via three Square activations:
            # stays in the sigmoid ACT table set (no ~1.3us table reload),
            # and with m = rowmean(e) the error exponent (e-m)^2/16 is
            # negligible.  p in (0.3, 3) fits fp8 with no max-subtraction.
            sum_e = smallp.tile([gb, 1], F32, tag="sume")
            nc.vector.tensor_reduce(sum_e[:], e_all[:], AxX, Add)
            bias_t = smallp.tile([gb, 1], F32, tag="bias")
            nc.vector.tensor_scalar(
                bias_t[:], sum_e[:], -1.0 / (N * 8 * WU_SCALE), 1.0, Mult, Add
            )
            y1 = smallp.tile([gb, N], F32, tag="y1")
            nc.scalar.activation(
                y1[:], e_all[:], Square, bias=bias_t[:, :1], scale=1.0 / (8 * WU_SCALE)
            )
            y2 = smallp.tile([gb, N], F32, tag="y2")
            nc.scalar.activation(y2[:], y1[:], Square)
            p_t = smallp.tile([gb, N], BF16, tag="p")
            sum_t = smallp.tile([gb, 1], F32, tag="sum")
            nc.scalar.activation(p_t[:], y2[:], Square, accum_out=sum_t[:, :1])
            r_t = smallp.tile([gb, 1], F32, tag="r")
            nc.vector.reciprocal(r_t[:], sum_t[:])

            # transpose p to n'-major for use as matmul weights
            pt_ps = pmix.tile([P, NCH_N, gb], F32, tag="o")
            for i in range(NCH_N):
                nc.tensor.matmul(
                    pt_ps[:, i, :], p_t[:, i * P : (i + 1) * P], idb_sb[:gb, :gb]
                )
            # gb real columns + 32 zero columns so each batch's lhsT slab
            # [bb : bb+32] is in-bounds (col 0 real, rest harmless)
            pt_sb = smallp.tile([P, NCH_N, gb + 32], BF16, tag="pt")
            nc.vector.memset(pt_sb[:, :, gb : gb + 32], 0.0)
            nc.vector.tensor_copy(pt_sb[:, :, 0:gb], pt_ps[:])
            outall = smallp.tile([gb, D], F32, tag="oall")
            return {"g": g, "gb": gb, "pt": pt_sb, "r": r_t, "outall": outall,
                    "o_ps": None, "xbs": None}

        def final_mms(ctx, bb, js):
            # pooling matvecs for prev-group batch bb, n'-chunks in `js`;
            # column group (bb+2)%4 — disjoint from the current e-matvec's.
            # (DoubleRow is illegal at dst partition base != 0, so these are
            # plain fp8 matmuls — fp8 streams at bf16 rate here.)
            qo = (bb + 2) % 4
            for i in js:
                nc.tensor.matmul(
                    ctx["o_ps"][qo * 32 : qo * 32 + 32, :],
                    ctx["pt"][:, i, bb : bb + 32],
                    ctx["xbs"][bb][:, i // 2, i % 2, :],
                    start=(i == 0),
                    stop=(i == NCH_N - 1),
                    tile_position=(0, qo * 32),
                )

        def final_drain(ctx, bb):
            # after each quad: PSUM -> SBUF, then permuted row gather
            # (row base+m sits at partition 32*((m+2)%4))
            o_sb = spool.tile([P, D], F32, tag="osb")
            nc.vector.tensor_copy(o_sb[:], ctx["o_ps"][:])
            base = bb - (bb % 4)
            nc.sync.dma_start(
                ctx["outall"][base : base + 2, :], o_sb[64 : 97 : 32, :]
            )
            nc.sync.dma_start(
                ctx["outall"][base + 2 : base + 4, :], o_sb[0 : 33 : 32, :]
            )

        def final_epilogue(ctx):
            outall, gb = ctx["outall"], ctx["gb"]
            nc.vector.tensor_scalar_mul(outall[:], outall[:], ctx["r"][:, :1])
            g0 = (ctx["g"] % nb_grp) * GRP
            nc.sync.dma_start(out_d.ap()[g0 : g0 + gb, :], outall[:])

        def compute_group(g, prev):
            gb = min(GRP, nb - (g % nb_grp) * GRP)
            assert gb % 4 == 0
            ctx = None
            if prev is not None:
                pg, pgb, pe_all, pxbs = prev
                ctx = final_prologue(pg, pgb, pe_all)
                ctx["xbs"] = pxbs
            e_all = smallp.tile([gb, N], F32, tag="eall")
            xbs = []
            xts = []
            e_ps = None
            for bb in range(gb):
                b = (g % nb_grp) * GRP + bb
                # four batches per DMA; natural layout on gpsimd (SWDGE, Pool
                # engine is otherwise idle), transposed on sync (HWDGE).
                if bb % 4 == 0:
                    xb4 = xnat.tile([P, 4, 2, 2, D], BF16, tag="xb")
                    nc.gpsimd.dma_start(
                        xb4[:],
                        x_d.ap()[b : b + 4].rearrange("b p h c d -> p b h c d"),
                    )
                    for k in range(4):
                        xbs.append(xb4[:, k])
                    xt4 = xtp.tile([P, 4, NCH_D, N], FP8, tag="xt")
                    nc.sync.dma_start(
                        xt4[:],
                        xt_d.ap()[b : b + 4].rearrange("b p c n -> p b c n"),
                    )
                    for k in range(4):
                        xts.append(xt4[:, k])
                xt = xts[bb]

                # x_u^T = W_u16^T @ XT (per d-chunk), then sigmoid with the
                # per-partition bias 16*xv (scale 1/16 undoes both)
                s_t = spool.tile([P, NCH_D, N], FP8, tag="s")
                for j in range(NCH_D):
                    xu = pxu.tile([P, N], F32, tag="xu")
                    nc.tensor.matmul(
                        xu[:],
                        wu_sb[:, :, j * P : (j + 1) * P],
                        xt[:, :, :],
                        start=True,
                        stop=True,
                        perf_mode=DR,
                    )
                    nc.scalar.activation(
                        s_t[:, j, :], xu[:], Sigmoid,
                        bias=xvb[:, j, b : b + 1], scale=1.0 / WU_SCALE,
                    )

                # 16*e[n'] = W_e16^T @ S : two fp8 matvecs landing on psum
                # partition 32*(bb%4).  Interleaved with the previous group's
                # pooling matvecs (disjoint column groups -> concurrent).
                q = bb % 4
                if q == 0:
                    e_ps = pe_p.tile([P, N], F32, tag="e")
                    if ctx is not None and bb < ctx["gb"]:
                        ctx["o_ps"] = pmix.tile([P, D], F32, name="o_ps", tag="o")
                for j in range(NCH_D):
                    nc.tensor.matmul(
                        e_ps[q * 32 : q * 32 + 32, :],
                        we_sb[:, j, :],
                        s_t[:, j, :],
                        start=(j == 0),
                        stop=(j == NCH_D - 1),
                        tile_position=(0, q * 32),
                    )
                    if ctx is not None and bb < ctx["gb"]:
                        final_mms(ctx, bb, (2 * j, 2 * j + 1))
                if ctx is not None and bb < ctx["gb"]:
                    if q == 3 or bb == ctx["gb"] - 1:
                        final_drain(ctx, bb)
                if q == 3 or bb == gb - 1:
                    # engines can't address strided partitions, but DMA can:
                    # PSUM -> SBUF copy (contiguous), then SBUF->SBUF gather
                    e_sb = spool.tile([P, N], F32, tag="esb")
                    nc.vector.tensor_copy(e_sb[:], e_ps[:])
                    nc.sync.dma_start(
                        e_all[bb - q : bb + 1, :], e_sb[0 : 32 * q + 1 : 32, :]
                    )
            if ctx is not None:
                for bb in range(gb, ctx["gb"]):  # leftover when prev group bigger
                    q = bb % 4
                    if q == 0:
                        ctx["o_ps"] = pmix.tile([P, D], F32, name="o_ps", tag="o")
                    final_mms(ctx, bb, range(NCH_N))
                    if q == 3 or bb == ctx["gb"] - 1:
                        final_drain(ctx, bb)
                final_epilogue(ctx)
            return gb, e_all, xbs

        def final_tail(prev):
            # the last group's final phase has no next group to hide in
            pg, pgb, pe_all, pxbs = prev
            ctx = final_prologue(pg, pgb, pe_all)
            ctx["xbs"] = pxbs
            for bb in range(pgb):
                q = bb % 4
                if q == 0:
                    ctx["o_ps"] = pmix.tile([P, D], F32, name="o_ps", tag="o")
                final_mms(ctx, bb, range(NCH_N))
                if q == 3 or bb == pgb - 1:
                    final_drain(ctx, bb)
            final_epilogue(ctx)

        # reps>1 repeats the whole sweep (for slope-based timing)
        nb_grp = (nb + GRP - 1) // GRP
        pending = None
        for g in range(nb_grp * reps):
            done = (g,) + compute_group(g, pending)
            pending = done
        final_tail(pending)

    nc.compile()
    return nc


_NC_CACHE = {}


def _get_nc(nb=BSH, reps=1):
    if (nb, reps) not in _NC_CACHE:
        _NC_CACHE[(nb, reps)] = build_nc(nb, reps)
    return _NC_CACHE[(nb, reps)]


def make_in_maps(x, last_nodes, W_u, b_u, W_v, W_e, ncores=NCORES):
    x = np.asarray(x, dtype=np.float32)
    last_nodes = np.asarray(last_nodes).astype(np.int64)
    W_u = np.asarray(W_u, dtype=np.float32)
    b_u = np.asarray(b_u, dtype=np.float32)
    W_v = np.asarray(W_v, dtype=np.float32)
    W_e = np.asarray(W_e, dtype=np.float32)

    bf = ml_dtypes.bfloat16
    f8 = ml_dtypes.float8_e4m3
    nb = x.shape[0] // ncores
    # natural fp8: xn[b, p, h, c, d] = x[b, 4p+2h+c, d] — a pure reshape
    xn = x.reshape(ncores, nb, P, 2, 2, D).astype(bf)
    # transposed fp8: xt[b, p, c, i*128+q] = x[b, 4q+i, c*128+p]
    xs = x.reshape(ncores, nb, N, D)
    xt = np.ascontiguousarray(
        xs.reshape(ncores, nb, P, NCH_N, NCH_D, P).transpose(0, 1, 5, 4, 3, 2)
    ).reshape(ncores, nb, P, NCH_D, N).astype(f8)
    ln = last_nodes.reshape(ncores, nb)
    offs = (np.arange(nb)[None, :] * N + ln).astype(np.int32).reshape(ncores, nb, 1)

    wu_h = np.ascontiguousarray(
        (W_u * WU_SCALE).reshape(NCH_D, P, D).transpose(1, 0, 2)
    ).astype(f8)
    wv_h = np.ascontiguousarray(
        (W_v * WU_SCALE).reshape(NCH_D, P, D).transpose(1, 0, 2)
    ).astype(bf)
    we_h = np.zeros((P, NCH_D, 32), dtype=f8)
    we_h[:, :, 0] = (W_e * WU_SCALE).reshape(NCH_D, P).T.astype(f8)
    bu_h = np.ascontiguousarray((b_u * WU_SCALE).reshape(NCH_D, P).T).astype(np.float32)
    idb = np.eye(P).astype(bf)

    return [
        {
            "x": np.ascontiguousarray(xn[c]),
            "xt": np.ascontiguousarray(xt[c]),
            "offs": offs[c],
            "wu": wu_h,
            "wv": wv_h,
            "we": we_h,
            "bu": bu_h,
            "idb": idb,
        }
        for c in range(ncores)
    ]


def kernel(x, last_nodes, W_u, b_u, W_v, W_e, **run_kwargs):
    nc = _get_nc(BSH)
    in_maps = make_in_maps(x, last_nodes, W_u, b_u, W_v, W_e)
    res = run_bass_kernel_spmd(nc, in_maps, core_ids=list(range(NCORES)), **run_kwargs)
    out = np.concatenate([r["out"] for r in res.results], axis=0).astype(np.float32)
    if run_kwargs:
        kernel.last_results = res
    return out
